# revision 6
# baseline (speedup 1.0000x reference)
"""GCNConv (N=20000, E=320000, D=1024) on 8 trn2 NeuronCores.

out = segment_sum(norm * h[col] -> row) with self-loops, h = x @ W^T + b
    = (segment_sum(norm * x[col] -> row)) @ W^T + s[row] * b,
      s[r] = sum of incoming norm (incl. self), norm = deg^-1/2 outer.

Aggregate-then-transform with an fp8 (e3m4) gather table: each core
stages the FULL x table pre-scaled to dis[c]*x[c]*2^k[c] (per-row pow2
normalization into e3m4's [7.75, 15.5] top range) in its local DRAM --
half the gather bytes of bf16, and e3m4's 4 mantissa bits keep the
end-to-end rel err ~1.3e-2.  Self loops ride along as ordinary edges
and duplicate sources within a (core, dest-block) are deduplicated, so
the selection matrices (built host-side, exact {1,2,...}*2^(kref-k)
values in e3m4) fold the whole normalization; the 2^-kref and dis[row]
factors fold into the PSUM->SBUF copy on the scalar engine.  Per dest
block (software-pipelined, gathers 2 blocks ahead, GEMM 1 block
behind): dma_gather source rows from the local fp8 table,
selection-matmul into fp32 PSUM, scaled-copy to fp16 on scalar,
PE-transpose z, then z @ W^T in fp16 with the bias applied on vector
as b*s + out.
"""

import numpy as np
import ml_dtypes

import concourse.bacc as bacc
import concourse.mybir as mybir
import concourse.tile as tile
from concourse import bass
from concourse import bass_utils
from concourse.masks import make_identity

N = 20000
E = 320000
D = 1024
NC = 8
NPC = N // NC            # 2500 real dest nodes per core
NBLK = 20                # dest blocks of 128 per core
NPCP = NBLK * 128        # 2560 padded dest nodes per core
P = 128
KT = D // P              # 8 contraction tiles of the out GEMM

_cache = {}
_pos = None

FP8 = ml_dtypes.float8_e3m4


def _preprocess(x, edge_index, W, b):
    x = np.asarray(x, dtype=np.float32)
    ei = np.asarray(edge_index)
    W = np.asarray(W, dtype=np.float32)
    b = np.asarray(b, dtype=np.float32)

    row = ei[0].astype(np.int64)
    col = ei[1].astype(np.int64)
    deg = (np.bincount(row, minlength=N) + 1).astype(np.float32)
    dis = deg ** -0.5

    # bias scale s[r] = total incoming weight incl. the dis^2 self loop
    wA = (dis[row] * dis[col]).astype(np.float64)
    s = (np.bincount(row, weights=wA, minlength=N) + dis * dis).astype(np.float32)

    # fp8 table: per-row pow2 normalization of v = dis[c] * x[c] into
    # e3m4's top binade; the inverse 2^(kref-k) goes into the selection
    # weights (exact in e3m4), the global 2^-kref into the z row scale
    v = dis[:, None] * x
    vmax = np.maximum(np.abs(v).max(axis=1), 1e-30)
    k = np.floor(np.log2(15.49 / vmax)).astype(np.int32)
    kref = int(k.max())
    xw8 = (v * (2.0 ** k)[:, None]).astype(FP8)
    selw = (2.0 ** (kref - k)).astype(np.float32)  # per-source sel weight

    # balance edge counts across the 160 (core, block) bins by dealing
    # nodes snake-wise in descending in-degree order: every bin ends up
    # with 125 nodes and an equal edge count, so the SPMD lockstep
    # schedule has no padding-tile overhead
    indeg = np.bincount(row, minlength=N)
    order_n = np.argsort(-indeg, kind="stable")
    NBIN = NC * NBLK
    bins = np.empty(N, dtype=np.int64)
    slot = np.empty(N, dtype=np.int64)
    fwd = np.arange(NBIN)
    for i in range(N // NBIN):
        selr = order_n[i * NBIN : (i + 1) * NBIN]
        bins[selr] = fwd if i % 2 == 0 else fwd[::-1]
        slot[selr] = i
    pos = (bins // NBLK) * NPCP + (bins % NBLK) * P + slot  # node -> padded row
    global _pos
    _pos = pos

    # append self loops as ordinary edges (weight folds the same way)
    rowA = np.concatenate([row, np.arange(N, dtype=np.int64)])
    colA = np.concatenate([col, np.arange(N, dtype=np.int64)])

    pdst = pos[rowA]
    core_of = pdst // NPCP
    within = pdst - core_of * NPCP
    blk = within // P
    dl_all = within % P  # dest offset in block

    # per (core, block) edge lists, sorted+deduped by source
    key = core_of * NBLK + blk
    order = np.argsort(key, kind="stable")
    ks = key[order]
    bounds = np.searchsorted(ks, np.arange(NC * NBLK + 1))
    colS, dlS = colA[order], dl_all[order]
    seg = {}
    cnt = np.zeros((NC, NBLK), dtype=np.int64)
    for c in range(NC):
        for bk in range(NBLK):
            i0, i1 = bounds[c * NBLK + bk], bounds[c * NBLK + bk + 1]
            srcs, sl = np.unique(colS[i0:i1], return_inverse=True)
            seg[(c, bk)] = (srcs, sl, dlS[i0:i1])
            cnt[c, bk] = len(srcs)

    # shared (SPMD lockstep) schedule: tiles per block = max over cores
    T_bs = tuple(int(-(-int(cnt[:, bk].max()) // P)) for bk in range(NBLK))
    NT = sum(T_bs)

    idx16 = np.zeros((NC, 16, NT * 8), dtype=np.int16)
    tstart = np.concatenate([[0], np.cumsum(T_bs)]).astype(np.int64)

    # GEMM rhs: 8 k-tiles of W^T; bias applied on vector as (b * s) + out
    WT8 = np.zeros((P, KT * D), dtype=np.float16)
    WTc = W.T.astype(np.float16)
    for kk in range(KT):
        WT8[:, kk * D : (kk + 1) * D] = WTc[kk * P : (kk + 1) * P, :]
    bbc = np.ascontiguousarray(
        np.broadcast_to(b.astype(np.float16), (P, D))
    )

    # per-node tables scattered into permuted padded positions
    dis_all = np.zeros(NC * NPCP, dtype=np.float32)
    dis_all[pos] = dis * (2.0 ** (-kref))
    s_all = np.zeros(NC * NPCP, dtype=np.float32)
    s_all[pos] = s

    in_maps = []
    selmax = 0.0
    for c in range(NC):
        sel = np.zeros((P, NT, P), dtype=np.float32)
        for bk in range(NBLK):
            t0 = int(tstart[bk])
            srcs, sl, dl = seg[(c, bk)]
            n = len(srcs)
            # slot s -> partition s%128, tile t0 + s//128 (gather layout)
            np.add.at(sel, (sl % P, t0 + sl // P, dl), selw[srcs][sl])
            buf = np.zeros(T_bs[bk] * P, dtype=np.int16)
            buf[:n] = srcs.astype(np.int16)
            idx16[c, :, t0 * 8 : (t0 + T_bs[bk]) * 8] = (
                buf.reshape(T_bs[bk] * 8, 16).T
            )
        selmax = max(selmax, float(sel.max()))
        sel8 = sel.astype(FP8)
        disT = dis_all[c * NPCP : (c + 1) * NPCP].reshape(NBLK, P).T
        sT = s_all[c * NPCP : (c + 1) * NPCP].reshape(NBLK, P).T
        in_maps.append(
            {
                "xw8": xw8,
                "disT": np.ascontiguousarray(disT),
                "sT": np.ascontiguousarray(sT),
                "WT8": np.ascontiguousarray(WT8),
                "bbc": bbc,
                "idx16": np.ascontiguousarray(np.tile(idx16[c], (8, 1))),
                "sel": np.ascontiguousarray(sel8.reshape(P, NT * P)),
            }
        )
    assert selmax <= 15.49, f"sel overflow {selmax}"
    return T_bs, NT, in_maps


def _build(T_bs, NT):
    f32 = mybir.dt.float32
    f16 = mybir.dt.float16
    f8 = mybir.dt.float8e3
    i16 = mybir.dt.int16
    tstart = {}
    t0 = 0
    for bk in range(NBLK):
        tstart[bk] = t0
        t0 += T_bs[bk]

    nc = bacc.Bacc("TRN2", target_bir_lowering=False, debug=False,
                   num_devices=NC, num_swdge_queues=4)
    xw8 = nc.dram_tensor("xw8", [N, D], f8, kind="ExternalInput").ap()
    disTi = nc.dram_tensor("disT", [P, NBLK], f32, kind="ExternalInput").ap()
    sTi = nc.dram_tensor("sT", [P, NBLK], f32, kind="ExternalInput").ap()
    WT8 = nc.dram_tensor("WT8", [P, KT * D], f16, kind="ExternalInput").ap()
    bbc = nc.dram_tensor("bbc", [P, D], f16, kind="ExternalInput").ap()
    idx16 = nc.dram_tensor("idx16", [P, NT * 8], i16, kind="ExternalInput").ap()
    seli = nc.dram_tensor("sel", [P, NT * P], f8, kind="ExternalInput").ap()
    yout = nc.dram_tensor("yout", [NPCP, D], f16, kind="ExternalOutput").ap()

    chunks = [slice(s0, s0 + 512) for s0 in range(0, D, 512)]

    with tile.TileContext(nc) as tc:
        with tc.tile_pool(name="const", bufs=1) as const:
            # gather-critical consts load first so the pipeline can start;
            # everything the gemm/bias needs is deferred until after the
            # first gathers are in flight (see below)
            ix_sb = const.tile([P, NT * 8], i16, name="ix_sb")
            nc.sync.dma_start(ix_sb[:], idx16[:])
            disT_sb = const.tile([P, NBLK], f32, name="disT_sb")
            ident = const.tile([P, P], f16, name="ident")
            wt_sb = const.tile([P, KT * D], f16, name="wt_sb")
            bbc_sb = const.tile([P, D], f16, name="bbc_sb")
            sT_sb = const.tile([P, NBLK], f32, name="sT_sb")

            def load_late_consts():
                nc.sync.dma_start(disT_sb[:], disTi[:])
                make_identity(nc, ident[:])
                for k in range(KT):
                    nc.sync.dma_start(wt_sb[:, k * D : (k + 1) * D],
                                      WT8[:, k * D : (k + 1) * D])
                nc.sync.dma_start(bbc_sb[:], bbc[:])
                nc.sync.dma_start(sT_sb[:], sTi[:])

            with tc.tile_pool(name="gath", bufs=3) as gp, \
                 tc.tile_pool(name="sel", bufs=4) as selp, \
                 tc.tile_pool(name="zps", bufs=2, space="PSUM") as zps, \
                 tc.tile_pool(name="tps", bufs=1, space="PSUM") as tps, \
                 tc.tile_pool(name="ops", bufs=1, space="PSUM") as ops, \
                 tc.tile_pool(name="zsb", bufs=3) as zsbp, \
                 tc.tile_pool(name="ztb", bufs=3) as ztbp, \
                 tc.tile_pool(name="aout", bufs=2) as aoutp:
                state = {"gq": 0}
                gs, sels, zds, zts = {}, {}, {}, {}

                # --- pipeline stages -------------------------------------
                def issue_sel(bk):
                    Tb, t0 = T_bs[bk], tstart[bk]
                    selb = selp.tile([P, Tb, P], f8, tag="selb")
                    nc.sync.dma_start(
                        selb[:], seli[:, t0 * P : (t0 + Tb) * P])
                    sels[bk] = selb

                def issue_gather(bk):
                    Tb, t0 = T_bs[bk], tstart[bk]
                    g = gp.tile([P, Tb, D], f8, tag="g")
                    GSUB = 8
                    for s0 in range(0, Tb, GSUB):
                        sn = min(GSUB, Tb - s0)
                        nc.gpsimd.dma_gather(
                            g[:, s0 : s0 + sn, :],
                            xw8[:],
                            ix_sb[:, (t0 + s0) * 8 : (t0 + s0 + sn) * 8],
                            sn * P,
                            sn * P,
                            D,
                            queue_num=state["gq"],
                            single_packet=True,
                        )
                        state["gq"] = (state["gq"] + 1) % 4
                    gs[bk] = g

                def issue_agg(bk):
                    # z[d, f] = sum over tiles of sel^T @ g, chunk-outer so
                    # each 512-col half finishes early and its scaled fp16
                    # copy (scalar engine, folds dis[dest] * 2^-kref) hides
                    # under the other half's matmuls
                    Tb = T_bs[bk]
                    g, selb = gs.pop(bk), sels.pop(bk)
                    halves = []
                    for ci, cs in enumerate(chunks):
                        zp = zps.tile([P, 512], f32, tag=f"zp{ci}")
                        for i in range(Tb):
                            nc.tensor.matmul(
                                zp[:], lhsT=selb[:, i, :], rhs=g[:, i, cs],
                                start=(i == 0), stop=(i == Tb - 1),
                            )
                        zsb = zsbp.tile([P, 512], f16, tag=f"zsb{ci}")
                        nc.scalar.activation(
                            out=zsb[:], in_=zp[:],
                            func=mybir.ActivationFunctionType.Copy,
                            scale=disT_sb[:, bk : bk + 1],
                        )
                        halves.append(zsb)
                    zds[bk] = halves

                def issue_transpose(bk):
                    # transpose z into 8 k-tiles [f, d] on the PE, in two
                    # halves so the zt copies overlap the other half
                    zh = zds.pop(bk)
                    parts = []
                    for ci in range(2):
                        tp = tps.tile([P, KT // 2, P], f16, tag=f"tp{ci}")
                        for ft in range(KT // 2):
                            nc.tensor.transpose(
                                tp[:, ft, :],
                                zh[ci][:, ft * P : (ft + 1) * P],
                                ident[:])
                        zt = ztbp.tile([P, KT // 2, P], f16, tag=f"zt{ci}")
                        nc.vector.tensor_copy(zt[:], tp[:])
                        parts.append(zt)
                    zts[bk] = parts

                def issue_gemm(bk):
                    # out = z @ W^T, bias on vector: ob = b * s + out
                    zt2 = zts.pop(bk)
                    op_ = ops.tile([P, D], f32)
                    for k in range(KT):
                        zt = zt2[k // 4]
                        for cs in chunks:
                            nc.tensor.matmul(
                                op_[:, cs], lhsT=zt[:, k % 4, :],
                                rhs=wt_sb[:, k * D + cs.start : k * D + cs.stop],
                                start=(k == 0), stop=(k == KT - 1),
                            )
                    ob = aoutp.tile([P, D], f16)
                    nc.vector.scalar_tensor_tensor(
                        out=ob[:], in0=bbc_sb[:], scalar=sT_sb[:, bk : bk + 1],
                        in1=op_[:], op0=mybir.AluOpType.mult,
                        op1=mybir.AluOpType.add,
                    )
                    nc.sync.dma_start(yout[bk * P : (bk + 1) * P, :], ob[:])

                # --- software pipeline: gathers/sels lead, gemm trails ---
                issue_sel(0)
                issue_sel(1)
                issue_gather(0)
                issue_gather(1)
                load_late_consts()
                for bk in range(NBLK):
                    if bk + 2 < NBLK:
                        issue_sel(bk + 2)
                        issue_gather(bk + 2)
                    issue_agg(bk)
                    if bk >= 1:
                        issue_gemm(bk - 1)
                    issue_transpose(bk)
                issue_gemm(NBLK - 1)

    nc.compile()
    return nc


def kernel(x, edge_index, W, b):
    T_bs, NT, in_maps = _preprocess(x, edge_index, W, b)
    key = (T_bs, NT)
    if key not in _cache:
        _cache[key] = _build(T_bs, NT)
    nc = _cache[key]
    res = bass_utils.run_bass_kernel_spmd(nc, in_maps, core_ids=list(range(NC)))
    stacked = np.concatenate([res.results[c]["yout"] for c in range(NC)], axis=0)
    return stacked[_pos].astype(np.float32)


# revision 11
# speedup vs baseline: 1.0041x; 1.0041x over previous
"""GCNConv (N=20000, E=320000, D=1024) on 8 trn2 NeuronCores.

out = segment_sum(norm * h[col] -> row) with self-loops, h = x @ W^T + b
    = (segment_sum(norm * x[col] -> row)) @ W^T + s[row] * b,
      s[r] = sum of incoming norm (incl. self), norm = deg^-1/2 outer.

Aggregate-then-transform with an fp8 (e3m4) gather table: each core
stages the FULL x table pre-scaled to dis[c]*x[c]*2^k[c] (per-row pow2
normalization into e3m4's [7.75, 15.5] top range) in its local DRAM --
half the gather bytes of bf16, and e3m4's 4 mantissa bits keep the
end-to-end rel err ~1.3e-2.  Self loops ride along as ordinary edges
and duplicate sources within a (core, dest-block) are deduplicated, so
the selection matrices (built host-side, exact {1,2,...}*2^(kref-k)
values in e3m4) fold the whole normalization; the 2^-kref and dis[row]
factors fold into the PSUM->SBUF copy on the scalar engine.  Per dest
block (software-pipelined, gathers 2 blocks ahead, GEMM 1 block
behind): dma_gather source rows from the local fp8 table,
selection-matmul into fp32 PSUM, scaled-copy to fp16 on scalar,
PE-transpose z, then z @ W^T in fp16 with the bias applied on vector
as b*s + out.
"""

import numpy as np
import ml_dtypes

import concourse.bacc as bacc
import concourse.mybir as mybir
import concourse.tile as tile
from concourse import bass
from concourse import bass_utils
from concourse.masks import make_identity

N = 20000
E = 320000
D = 1024
NC = 8
NPC = N // NC            # 2500 real dest nodes per core
NBLK = 20                # dest blocks of 128 per core
NPCP = NBLK * 128        # 2560 padded dest nodes per core
P = 128
KT = D // P              # 8 contraction tiles of the out GEMM

_cache = {}
_pos = None

FP8 = ml_dtypes.float8_e3m4


def _preprocess(x, edge_index, W, b):
    x = np.asarray(x, dtype=np.float32)
    ei = np.asarray(edge_index)
    W = np.asarray(W, dtype=np.float32)
    b = np.asarray(b, dtype=np.float32)

    row = ei[0].astype(np.int64)
    col = ei[1].astype(np.int64)
    deg = (np.bincount(row, minlength=N) + 1).astype(np.float32)
    dis = deg ** -0.5

    # bias scale s[r] = total incoming weight incl. the dis^2 self loop
    wA = (dis[row] * dis[col]).astype(np.float64)
    s = (np.bincount(row, weights=wA, minlength=N) + dis * dis).astype(np.float32)

    # fp8 table: per-row pow2 normalization of v = dis[c] * x[c] into
    # e3m4's top binade; the inverse 2^(kref-k) goes into the selection
    # weights (exact in e3m4), the global 2^-kref into the z row scale
    v = dis[:, None] * x
    vmax = np.maximum(np.abs(v).max(axis=1), 1e-30)
    k = np.floor(np.log2(15.49 / vmax)).astype(np.int32)
    kref = int(k.max())
    xw8 = (v * (2.0 ** k)[:, None]).astype(FP8)
    selw = (2.0 ** (kref - k)).astype(np.float32)  # per-source sel weight

    # balance edge counts across the 160 (core, block) bins by dealing
    # nodes snake-wise in descending in-degree order: every bin ends up
    # with 125 nodes and an equal edge count, so the SPMD lockstep
    # schedule has no padding-tile overhead
    indeg = np.bincount(row, minlength=N)
    order_n = np.argsort(-indeg, kind="stable")
    NBIN = NC * NBLK
    bins = np.empty(N, dtype=np.int64)
    slot = np.empty(N, dtype=np.int64)
    fwd = np.arange(NBIN)
    for i in range(N // NBIN):
        selr = order_n[i * NBIN : (i + 1) * NBIN]
        bins[selr] = fwd if i % 2 == 0 else fwd[::-1]
        slot[selr] = i
    pos = (bins // NBLK) * NPCP + (bins % NBLK) * P + slot  # node -> padded row
    global _pos
    _pos = pos

    # append self loops as ordinary edges (weight folds the same way)
    rowA = np.concatenate([row, np.arange(N, dtype=np.int64)])
    colA = np.concatenate([col, np.arange(N, dtype=np.int64)])

    pdst = pos[rowA]
    core_of = pdst // NPCP
    within = pdst - core_of * NPCP
    blk = within // P
    dl_all = within % P  # dest offset in block

    # per (core, block) edge lists, sorted+deduped by source
    key = core_of * NBLK + blk
    order = np.argsort(key, kind="stable")
    ks = key[order]
    bounds = np.searchsorted(ks, np.arange(NC * NBLK + 1))
    colS, dlS = colA[order], dl_all[order]
    seg = {}
    cnt = np.zeros((NC, NBLK), dtype=np.int64)
    for c in range(NC):
        for bk in range(NBLK):
            i0, i1 = bounds[c * NBLK + bk], bounds[c * NBLK + bk + 1]
            srcs, sl = np.unique(colS[i0:i1], return_inverse=True)
            seg[(c, bk)] = (srcs, sl, dlS[i0:i1])
            cnt[c, bk] = len(srcs)

    # shared (SPMD lockstep) schedule: tiles per block = max over cores
    T_bs = tuple(int(-(-int(cnt[:, bk].max()) // P)) for bk in range(NBLK))
    NT = sum(T_bs)

    idx16 = np.zeros((NC, 16, NT * 8), dtype=np.int16)
    tstart = np.concatenate([[0], np.cumsum(T_bs)]).astype(np.int64)

    # GEMM rhs: 8 k-tiles of W^T; bias applied on vector as (b * s) + out
    WT8 = np.zeros((P, KT * D), dtype=np.float16)
    WTc = W.T.astype(np.float16)
    for kk in range(KT):
        WT8[:, kk * D : (kk + 1) * D] = WTc[kk * P : (kk + 1) * P, :]
    bbc = np.ascontiguousarray(
        np.broadcast_to(b.astype(np.float16), (P, D))
    )

    # per-node tables scattered into permuted padded positions
    dis_all = np.zeros(NC * NPCP, dtype=np.float32)
    dis_all[pos] = dis * (2.0 ** (-kref))
    s_all = np.zeros(NC * NPCP, dtype=np.float32)
    s_all[pos] = s

    in_maps = []
    selmax = 0.0
    for c in range(NC):
        sel = np.zeros((P, NT, P), dtype=np.float32)
        for bk in range(NBLK):
            t0 = int(tstart[bk])
            srcs, sl, dl = seg[(c, bk)]
            n = len(srcs)
            # slot s -> partition s%128, tile t0 + s//128 (gather layout)
            np.add.at(sel, (sl % P, t0 + sl // P, dl), selw[srcs][sl])
            buf = np.zeros(T_bs[bk] * P, dtype=np.int16)
            buf[:n] = srcs.astype(np.int16)
            idx16[c, :, t0 * 8 : (t0 + T_bs[bk]) * 8] = (
                buf.reshape(T_bs[bk] * 8, 16).T
            )
        selmax = max(selmax, float(sel.max()))
        sel8 = sel.astype(FP8)
        disT = dis_all[c * NPCP : (c + 1) * NPCP].reshape(NBLK, P).T
        sT = s_all[c * NPCP : (c + 1) * NPCP].reshape(NBLK, P).T
        in_maps.append(
            {
                "xw8": xw8,
                "disT": np.ascontiguousarray(disT),
                "sT": np.ascontiguousarray(sT),
                "WT8": np.ascontiguousarray(WT8),
                "bbc": bbc,
                "idx16": np.ascontiguousarray(np.tile(idx16[c], (8, 1))),
                "sel": np.ascontiguousarray(sel8.reshape(P, NT * P)),
            }
        )
    assert selmax <= 15.49, f"sel overflow {selmax}"
    return T_bs, NT, in_maps


def _build(T_bs, NT):
    f32 = mybir.dt.float32
    f16 = mybir.dt.float16
    f8 = mybir.dt.float8e3
    i16 = mybir.dt.int16
    tstart = {}
    t0 = 0
    for bk in range(NBLK):
        tstart[bk] = t0
        t0 += T_bs[bk]

    nc = bacc.Bacc("TRN2", target_bir_lowering=False, debug=False,
                   num_devices=NC, num_swdge_queues=4)
    xw8 = nc.dram_tensor("xw8", [N, D], f8, kind="ExternalInput").ap()
    disTi = nc.dram_tensor("disT", [P, NBLK], f32, kind="ExternalInput").ap()
    sTi = nc.dram_tensor("sT", [P, NBLK], f32, kind="ExternalInput").ap()
    WT8 = nc.dram_tensor("WT8", [P, KT * D], f16, kind="ExternalInput").ap()
    bbc = nc.dram_tensor("bbc", [P, D], f16, kind="ExternalInput").ap()
    idx16 = nc.dram_tensor("idx16", [P, NT * 8], i16, kind="ExternalInput").ap()
    seli = nc.dram_tensor("sel", [P, NT * P], f8, kind="ExternalInput").ap()
    yout = nc.dram_tensor("yout", [NPCP, D], f16, kind="ExternalOutput").ap()

    chunks = [slice(s0, s0 + 512) for s0 in range(0, D, 512)]

    with tile.TileContext(nc) as tc:
        with tc.tile_pool(name="const", bufs=1) as const:
            # gather-critical consts load first so the pipeline can start;
            # everything the gemm/bias needs is deferred until after the
            # first gathers are in flight (see below)
            ix_sb = const.tile([P, NT * 8], i16, name="ix_sb")
            disT_sb = const.tile([P, NBLK], f32, name="disT_sb")
            ident = const.tile([P, P], f16, name="ident")
            wt_sb = const.tile([P, KT * D], f16, name="wt_sb")
            bbc_sb = const.tile([P, D], f16, name="bbc_sb")
            sT_sb = const.tile([P, NBLK], f32, name="sT_sb")

            def load_ix(bk):
                # per-block index slices: gather(bk) waits only on its slice
                t0, Tb = tstart[bk], T_bs[bk]
                nc.sync.dma_start(ix_sb[:, t0 * 8 : (t0 + Tb) * 8],
                                  idx16[:, t0 * 8 : (t0 + Tb) * 8])

            def load_late_consts():
                # big consts ride the scalar HWDGE ring so they never sit
                # ahead of the gather-critical ix/sel loads on sync
                nc.scalar.dma_start(disT_sb[:], disTi[:])
                make_identity(nc, ident[:])
                for k in range(KT):
                    nc.scalar.dma_start(wt_sb[:, k * D : (k + 1) * D],
                                        WT8[:, k * D : (k + 1) * D])
                nc.scalar.dma_start(bbc_sb[:], bbc[:])
                nc.scalar.dma_start(sT_sb[:], sTi[:])

            with tc.tile_pool(name="gath", bufs=3) as gp, \
                 tc.tile_pool(name="sel", bufs=4) as selp, \
                 tc.tile_pool(name="zps", bufs=2, space="PSUM") as zps, \
                 tc.tile_pool(name="tps", bufs=1, space="PSUM") as tps, \
                 tc.tile_pool(name="ops", bufs=1, space="PSUM") as ops, \
                 tc.tile_pool(name="zsb", bufs=3) as zsbp, \
                 tc.tile_pool(name="ztb", bufs=3) as ztbp, \
                 tc.tile_pool(name="aout", bufs=2) as aoutp:
                state = {"gq": 0}
                gs, sels, zds, zts = {}, {}, {}, {}

                # --- pipeline stages -------------------------------------
                def issue_sel(bk):
                    Tb, t0 = T_bs[bk], tstart[bk]
                    selb = selp.tile([P, Tb, P], f8, tag="selb")
                    nc.sync.dma_start(
                        selb[:], seli[:, t0 * P : (t0 + Tb) * P])
                    sels[bk] = selb

                def issue_gather(bk):
                    Tb, t0 = T_bs[bk], tstart[bk]
                    g = gp.tile([P, Tb, D], f8, tag="g")
                    GSUB = 8
                    for s0 in range(0, Tb, GSUB):
                        sn = min(GSUB, Tb - s0)
                        nc.gpsimd.dma_gather(
                            g[:, s0 : s0 + sn, :],
                            xw8[:],
                            ix_sb[:, (t0 + s0) * 8 : (t0 + s0 + sn) * 8],
                            sn * P,
                            sn * P,
                            D,
                            queue_num=state["gq"],
                            single_packet=True,
                        )
                        state["gq"] = (state["gq"] + 1) % 4
                    gs[bk] = g

                def issue_agg(bk):
                    # z[d, f] = sum over tiles of sel^T @ g, chunk-outer so
                    # each 512-col half finishes early and its scaled fp16
                    # copy (scalar engine, folds dis[dest] * 2^-kref) hides
                    # under the other half's matmuls
                    Tb = T_bs[bk]
                    g, selb = gs.pop(bk), sels.pop(bk)
                    halves = []
                    for ci, cs in enumerate(chunks):
                        zp = zps.tile([P, 512], f32, tag=f"zp{ci}")
                        for i in range(Tb):
                            nc.tensor.matmul(
                                zp[:], lhsT=selb[:, i, :], rhs=g[:, i, cs],
                                start=(i == 0), stop=(i == Tb - 1),
                            )
                        zsb = zsbp.tile([P, 512], f16, tag=f"zsb{ci}")
                        if ci == 0:
                            # chunk 0 copy hides under chunk 1's matmuls
                            nc.scalar.activation(
                                out=zsb[:], in_=zp[:],
                                func=mybir.ActivationFunctionType.Copy,
                                scale=disT_sb[:, bk : bk + 1],
                            )
                        else:
                            # chunk 1 is on the critical path to the
                            # transposes -- vector is faster
                            nc.vector.tensor_scalar_mul(
                                out=zsb[:], in0=zp[:],
                                scalar1=disT_sb[:, bk : bk + 1],
                            )
                        halves.append(zsb)
                    zds[bk] = halves

                def issue_transpose(bk):
                    # transpose z into 8 k-tiles [f, d] on the PE, in two
                    # halves so the zt copies overlap the other half
                    zh = zds.pop(bk)
                    parts = []
                    for ci in range(2):
                        tp = tps.tile([P, KT // 2, P], f16, tag=f"tp{ci}")
                        for ft in range(KT // 2):
                            nc.tensor.transpose(
                                tp[:, ft, :],
                                zh[ci][:, ft * P : (ft + 1) * P],
                                ident[:])
                        zt = ztbp.tile([P, KT // 2, P], f16, tag=f"zt{ci}")
                        nc.vector.tensor_copy(zt[:], tp[:])
                        parts.append(zt)
                    zts[bk] = parts

                def issue_gemm(bk):
                    # out = z @ W^T, chunk-outer so each 512-col half gets
                    # its bias (vector: ob = b * s + out) and store while
                    # the other half's matmuls run
                    zt2 = zts.pop(bk)
                    for ci, cs in enumerate(chunks):
                        op_ = ops.tile([P, 512], f32, tag=f"op{ci}")
                        for k in range(KT):
                            zt = zt2[k // 4]
                            nc.tensor.matmul(
                                op_[:], lhsT=zt[:, k % 4, :],
                                rhs=wt_sb[:, k * D + cs.start : k * D + cs.stop],
                                start=(k == 0), stop=(k == KT - 1),
                            )
                        ob = aoutp.tile([P, 512], f16, tag=f"ob{ci}")
                        nc.vector.scalar_tensor_tensor(
                            out=ob[:], in0=bbc_sb[:, cs],
                            scalar=sT_sb[:, bk : bk + 1],
                            in1=op_[:], op0=mybir.AluOpType.mult,
                            op1=mybir.AluOpType.add,
                        )
                        nc.sync.dma_start(yout[bk * P : (bk + 1) * P, cs], ob[:])

                # --- software pipeline: gathers/sels lead, gemm trails ---
                load_ix(0)
                load_ix(1)
                issue_sel(0)
                issue_sel(1)
                issue_gather(0)
                issue_gather(1)
                load_late_consts()
                for bk in range(NBLK):
                    if bk + 2 < NBLK:
                        load_ix(bk + 2)
                        issue_sel(bk + 2)
                        issue_gather(bk + 2)
                    issue_agg(bk)
                    if bk >= 1:
                        issue_gemm(bk - 1)
                    issue_transpose(bk)
                issue_gemm(NBLK - 1)

    nc.compile()
    return nc


def kernel(x, edge_index, W, b):
    T_bs, NT, in_maps = _preprocess(x, edge_index, W, b)
    key = (T_bs, NT)
    if key not in _cache:
        _cache[key] = _build(T_bs, NT)
    nc = _cache[key]
    res = bass_utils.run_bass_kernel_spmd(nc, in_maps, core_ids=list(range(NC)))
    stacked = np.concatenate([res.results[c]["yout"] for c in range(NC)], axis=0)
    return stacked[_pos].astype(np.float32)


# revision 15
# speedup vs baseline: 1.0121x; 1.0080x over previous
"""GCNConv (N=20000, E=320000, D=1024) on 8 trn2 NeuronCores.

out = segment_sum(norm * h[col] -> row) with self-loops, h = x @ W^T + b
    = (segment_sum(norm * x[col] -> row)) @ W^T + s[row] * b,
      s[r] = sum of incoming norm (incl. self), norm = deg^-1/2 outer.

Aggregate-then-transform with an fp8 (e3m4) gather table: each core
stages the FULL x table pre-scaled to dis[c]*x[c]*2^k[c] (per-row pow2
normalization into e3m4's [7.75, 15.5] top range) in its local DRAM --
half the gather bytes of bf16, and e3m4's 4 mantissa bits keep the
end-to-end rel err ~1.3e-2.  Self loops ride along as ordinary edges
and duplicate sources within a (core, dest-block) are deduplicated, so
the selection matrices (built host-side, exact {1,2,...}*2^(kref-k)
values in e3m4) fold the whole normalization; the 2^-kref and dis[row]
factors fold into the PSUM->SBUF copy on the scalar engine.  Per dest
block (software-pipelined, gathers 2 blocks ahead, GEMM 1 block
behind): dma_gather source rows from the local fp8 table,
selection-matmul into fp32 PSUM, scaled-copy to fp16 on scalar,
PE-transpose z, then z @ W^T in fp16 with the bias applied on vector
as b*s + out.
"""

import numpy as np
import ml_dtypes

import concourse.bacc as bacc
import concourse.mybir as mybir
import concourse.tile as tile
from concourse import bass
from concourse import bass_utils
from concourse.masks import make_identity

N = 20000
E = 320000
D = 1024
NC = 8
NPC = N // NC            # 2500 real dest nodes per core
NBLK = 20                # dest blocks of 128 per core
NPCP = NBLK * 128        # 2560 padded dest nodes per core
P = 128
KT = D // P              # 8 contraction tiles of the out GEMM

_cache = {}
_pos = None

FP8 = ml_dtypes.float8_e3m4


def _preprocess(x, edge_index, W, b):
    x = np.asarray(x, dtype=np.float32)
    ei = np.asarray(edge_index)
    W = np.asarray(W, dtype=np.float32)
    b = np.asarray(b, dtype=np.float32)

    row = ei[0].astype(np.int64)
    col = ei[1].astype(np.int64)
    deg = (np.bincount(row, minlength=N) + 1).astype(np.float32)
    dis = deg ** -0.5

    # bias scale s[r] = total incoming weight incl. the dis^2 self loop
    wA = (dis[row] * dis[col]).astype(np.float64)
    s = (np.bincount(row, weights=wA, minlength=N) + dis * dis).astype(np.float32)

    # fp8 table: per-row pow2 normalization of v = dis[c] * x[c] into
    # e3m4's top binade; the inverse 2^(kref-k) goes into the selection
    # weights (exact in e3m4), the global 2^-kref into the z row scale
    v = dis[:, None] * x
    vmax = np.maximum(np.abs(v).max(axis=1), 1e-30)
    k = np.floor(np.log2(15.49 / vmax)).astype(np.int32)
    kref = int(k.max())
    xw8 = (v * (2.0 ** k)[:, None]).astype(FP8)
    selw = (2.0 ** (kref - k)).astype(np.float32)  # per-source sel weight

    # balance edge counts across the 160 (core, block) bins by dealing
    # nodes snake-wise in descending in-degree order: every bin ends up
    # with 125 nodes and an equal edge count, so the SPMD lockstep
    # schedule has no padding-tile overhead
    indeg = np.bincount(row, minlength=N)
    order_n = np.argsort(-indeg, kind="stable")
    NBIN = NC * NBLK
    bins = np.empty(N, dtype=np.int64)
    slot = np.empty(N, dtype=np.int64)
    fwd = np.arange(NBIN)
    for i in range(N // NBIN):
        selr = order_n[i * NBIN : (i + 1) * NBIN]
        bins[selr] = fwd if i % 2 == 0 else fwd[::-1]
        slot[selr] = i
    pos = (bins // NBLK) * NPCP + (bins % NBLK) * P + slot  # node -> padded row
    global _pos
    _pos = pos

    # append self loops as ordinary edges (weight folds the same way)
    rowA = np.concatenate([row, np.arange(N, dtype=np.int64)])
    colA = np.concatenate([col, np.arange(N, dtype=np.int64)])

    pdst = pos[rowA]
    core_of = pdst // NPCP
    within = pdst - core_of * NPCP
    blk = within // P
    dl_all = within % P  # dest offset in block

    # per (core, block) edge lists, sorted+deduped by source
    key = core_of * NBLK + blk
    order = np.argsort(key, kind="stable")
    ks = key[order]
    bounds = np.searchsorted(ks, np.arange(NC * NBLK + 1))
    colS, dlS = colA[order], dl_all[order]
    seg = {}
    cnt = np.zeros((NC, NBLK), dtype=np.int64)
    for c in range(NC):
        for bk in range(NBLK):
            i0, i1 = bounds[c * NBLK + bk], bounds[c * NBLK + bk + 1]
            srcs, sl = np.unique(colS[i0:i1], return_inverse=True)
            seg[(c, bk)] = (srcs, sl, dlS[i0:i1])
            cnt[c, bk] = len(srcs)

    # shared (SPMD lockstep) schedule: tiles per block = max over cores
    T_bs = tuple(int(-(-int(cnt[:, bk].max()) // P)) for bk in range(NBLK))
    NT = sum(T_bs)

    idx16 = np.zeros((NC, 16, NT * 8), dtype=np.int16)
    tstart = np.concatenate([[0], np.cumsum(T_bs)]).astype(np.int64)

    # GEMM rhs: 8 k-tiles of W^T; bias applied on vector as (b * s) + out
    WT8 = np.zeros((P, KT * D), dtype=np.float16)
    WTc = W.T.astype(np.float16)
    for kk in range(KT):
        WT8[:, kk * D : (kk + 1) * D] = WTc[kk * P : (kk + 1) * P, :]
    bbc = np.ascontiguousarray(
        np.broadcast_to(b.astype(np.float16), (P, D))
    )

    # per-node tables scattered into permuted padded positions
    dis_all = np.zeros(NC * NPCP, dtype=np.float32)
    dis_all[pos] = dis * (2.0 ** (-kref))
    s_all = np.zeros(NC * NPCP, dtype=np.float32)
    s_all[pos] = s

    in_maps = []
    selmax = 0.0
    for c in range(NC):
        sel = np.zeros((P, NT, P), dtype=np.float32)
        for bk in range(NBLK):
            t0 = int(tstart[bk])
            srcs, sl, dl = seg[(c, bk)]
            n = len(srcs)
            # slot s -> partition s%128, tile t0 + s//128 (gather layout)
            np.add.at(sel, (sl % P, t0 + sl // P, dl), selw[srcs][sl])
            buf = np.zeros(T_bs[bk] * P, dtype=np.int16)
            buf[:n] = srcs.astype(np.int16)
            idx16[c, :, t0 * 8 : (t0 + T_bs[bk]) * 8] = (
                buf.reshape(T_bs[bk] * 8, 16).T
            )
        selmax = max(selmax, float(sel.max()))
        sel8 = sel.astype(FP8)
        disT = dis_all[c * NPCP : (c + 1) * NPCP].reshape(NBLK, P).T
        sT = s_all[c * NPCP : (c + 1) * NPCP].reshape(NBLK, P).T
        in_maps.append(
            {
                "xw8": xw8,
                "disT": np.ascontiguousarray(disT),
                "sT": np.ascontiguousarray(sT),
                "WT8": np.ascontiguousarray(WT8),
                "bbc": bbc,
                "idx16": np.ascontiguousarray(np.tile(idx16[c], (8, 1))),
                "sel": np.ascontiguousarray(sel8.reshape(P, NT * P)),
            }
        )
    assert selmax <= 15.49, f"sel overflow {selmax}"
    return T_bs, NT, in_maps


def _build(T_bs, NT):
    f32 = mybir.dt.float32
    f16 = mybir.dt.float16
    f8 = mybir.dt.float8e3
    i16 = mybir.dt.int16
    tstart = {}
    t0 = 0
    for bk in range(NBLK):
        tstart[bk] = t0
        t0 += T_bs[bk]

    nc = bacc.Bacc("TRN2", target_bir_lowering=False, debug=False,
                   num_devices=NC, num_swdge_queues=4)
    xw8 = nc.dram_tensor("xw8", [N, D], f8, kind="ExternalInput").ap()
    disTi = nc.dram_tensor("disT", [P, NBLK], f32, kind="ExternalInput").ap()
    sTi = nc.dram_tensor("sT", [P, NBLK], f32, kind="ExternalInput").ap()
    WT8 = nc.dram_tensor("WT8", [P, KT * D], f16, kind="ExternalInput").ap()
    bbc = nc.dram_tensor("bbc", [P, D], f16, kind="ExternalInput").ap()
    idx16 = nc.dram_tensor("idx16", [P, NT * 8], i16, kind="ExternalInput").ap()
    seli = nc.dram_tensor("sel", [P, NT * P], f8, kind="ExternalInput").ap()
    yout = nc.dram_tensor("yout", [NPCP, D], f16, kind="ExternalOutput").ap()

    chunks = [slice(s0, s0 + 512) for s0 in range(0, D, 512)]

    with tile.TileContext(nc) as tc:
        with tc.tile_pool(name="const", bufs=1) as const:
            # gather-critical consts load first so the pipeline can start;
            # everything the gemm/bias needs is deferred until after the
            # first gathers are in flight (see below)
            ix_sb = const.tile([P, NT * 8], i16, name="ix_sb")
            disT_sb = const.tile([P, NBLK], f32, name="disT_sb")
            ident = const.tile([P, P], f16, name="ident")
            wt_sb = const.tile([P, KT * D], f16, name="wt_sb")
            bbc_sb = const.tile([P, D], f16, name="bbc_sb")
            sT_sb = const.tile([P, NBLK], f32, name="sT_sb")

            def load_ix(bk, eng=None):
                # per-block index slices: gather(bk) waits only on its slice
                t0, Tb = tstart[bk], T_bs[bk]
                (eng or nc.sync).dma_start(ix_sb[:, t0 * 8 : (t0 + Tb) * 8],
                                           idx16[:, t0 * 8 : (t0 + Tb) * 8])

            def load_late_consts():
                # big consts ride the scalar HWDGE ring so they never sit
                # ahead of the gather-critical ix/sel loads on sync
                nc.scalar.dma_start(disT_sb[:], disTi[:])
                for k in range(KT):
                    nc.scalar.dma_start(wt_sb[:, k * D : (k + 1) * D],
                                        WT8[:, k * D : (k + 1) * D])
                nc.scalar.dma_start(bbc_sb[:], bbc[:])
                nc.scalar.dma_start(sT_sb[:], sTi[:])

            with tc.tile_pool(name="gath", bufs=3) as gp, \
                 tc.tile_pool(name="sel", bufs=4) as selp, \
                 tc.tile_pool(name="zps", bufs=2, space="PSUM") as zps, \
                 tc.tile_pool(name="tps", bufs=1, space="PSUM") as tps, \
                 tc.tile_pool(name="ops", bufs=1, space="PSUM") as ops, \
                 tc.tile_pool(name="zsb", bufs=3) as zsbp, \
                 tc.tile_pool(name="ztb", bufs=3) as ztbp, \
                 tc.tile_pool(name="aout", bufs=2) as aoutp:
                state = {"gq": 0}
                gs, sels, zds, zts = {}, {}, {}, {}

                # --- pipeline stages -------------------------------------
                def issue_sel(bk):
                    Tb, t0 = T_bs[bk], tstart[bk]
                    selb = selp.tile([P, Tb, P], f8, tag="selb")
                    nc.sync.dma_start(
                        selb[:], seli[:, t0 * P : (t0 + Tb) * P])
                    sels[bk] = selb

                def issue_gather(bk, splits=(8, 8)):
                    Tb, t0 = T_bs[bk], tstart[bk]
                    g = gp.tile([P, Tb, D], f8, tag="g")
                    bounds = [0]
                    for s in splits:
                        bounds.append(min(Tb, bounds[-1] + s))
                    while bounds[-1] < Tb:
                        bounds.append(min(Tb, bounds[-1] + 8))
                    for s0, s1 in zip(bounds, bounds[1:]):
                        sn = s1 - s0
                        if sn <= 0:
                            continue
                        nc.gpsimd.dma_gather(
                            g[:, s0 : s0 + sn, :],
                            xw8[:],
                            ix_sb[:, (t0 + s0) * 8 : (t0 + s0 + sn) * 8],
                            sn * P,
                            sn * P,
                            D,
                            queue_num=state["gq"],
                            single_packet=True,
                        )
                        state["gq"] = (state["gq"] + 1) % 4
                    gs[bk] = g

                def issue_agg(bk):
                    # z[d, f] = sum over tiles of sel^T @ g, chunk-outer so
                    # each 512-col half finishes early and its scaled fp16
                    # copy (scalar engine, folds dis[dest] * 2^-kref) hides
                    # under the other half's matmuls
                    Tb = T_bs[bk]
                    g, selb = gs.pop(bk), sels.pop(bk)
                    halves = []
                    for ci, cs in enumerate(chunks):
                        zp = zps.tile([P, 512], f32, tag=f"zp{ci}")
                        for i in range(Tb):
                            nc.tensor.matmul(
                                zp[:], lhsT=selb[:, i, :], rhs=g[:, i, cs],
                                start=(i == 0), stop=(i == Tb - 1),
                            )
                        zsb = zsbp.tile([P, 512], f16, tag=f"zsb{ci}")
                        if ci == 0:
                            # chunk 0 copy hides under chunk 1's matmuls
                            nc.scalar.activation(
                                out=zsb[:], in_=zp[:],
                                func=mybir.ActivationFunctionType.Copy,
                                scale=disT_sb[:, bk : bk + 1],
                            )
                        else:
                            # chunk 1 is on the critical path to the
                            # transposes -- vector is faster
                            nc.vector.tensor_scalar_mul(
                                out=zsb[:], in0=zp[:],
                                scalar1=disT_sb[:, bk : bk + 1],
                            )
                        halves.append(zsb)
                    zds[bk] = halves

                def issue_transpose(bk):
                    # transpose z into 8 k-tiles [f, d] on the PE, in two
                    # halves so the zt copies overlap the other half
                    zh = zds.pop(bk)
                    parts = []
                    for ci in range(2):
                        tp = tps.tile([P, KT // 2, P], f16, tag=f"tp{ci}")
                        for ft in range(KT // 2):
                            nc.tensor.transpose(
                                tp[:, ft, :],
                                zh[ci][:, ft * P : (ft + 1) * P],
                                ident[:])
                        zt = ztbp.tile([P, KT // 2, P], f16, tag=f"zt{ci}")
                        nc.vector.tensor_copy(zt[:], tp[:])
                        parts.append(zt)
                    zts[bk] = parts

                def issue_gemm(bk):
                    # out = z @ W^T, chunk-outer so each 512-col half gets
                    # its bias (vector: ob = b * s + out) and store while
                    # the other half's matmuls run
                    zt2 = zts.pop(bk)
                    for ci, cs in enumerate(chunks):
                        op_ = ops.tile([P, 512], f32, tag=f"op{ci}")
                        for k in range(KT):
                            zt = zt2[k // 4]
                            nc.tensor.matmul(
                                op_[:], lhsT=zt[:, k % 4, :],
                                rhs=wt_sb[:, k * D + cs.start : k * D + cs.stop],
                                start=(k == 0), stop=(k == KT - 1),
                            )
                        ob = aoutp.tile([P, 512], f16, tag=f"ob{ci}")
                        nc.vector.scalar_tensor_tensor(
                            out=ob[:], in0=bbc_sb[:, cs],
                            scalar=sT_sb[:, bk : bk + 1],
                            in1=op_[:], op0=mybir.AluOpType.mult,
                            op1=mybir.AluOpType.add,
                        )
                        nc.sync.dma_start(yout[bk * P : (bk + 1) * P, cs], ob[:])

                # --- software pipeline: gathers/sels lead, gemm trails ---
                # first two ix loads ride SWDGE (HWDGE's first completion is
                # slow at startup); small first gather call lands data early
                load_ix(0, eng=nc.gpsimd)
                load_ix(1, eng=nc.gpsimd)
                issue_sel(0)
                issue_sel(1)
                make_identity(nc, ident[:])
                issue_gather(0, splits=(4, 4, 8))
                issue_gather(1)
                load_late_consts()
                # keep the PE HAM clock-gate warm while gathers land
                warm = zps.tile([P, 512], f32, tag="zp0", name="warm")
                for _ in range(48):
                    nc.tensor.matmul(warm[:, :P], lhsT=ident[:], rhs=ident[:],
                                     start=True, stop=True)
                for bk in range(NBLK):
                    if bk + 2 < NBLK:
                        load_ix(bk + 2)
                        issue_sel(bk + 2)
                        issue_gather(bk + 2)
                    issue_agg(bk)
                    if bk >= 1:
                        issue_gemm(bk - 1)
                    issue_transpose(bk)
                issue_gemm(NBLK - 1)

    nc.compile()
    return nc


def kernel(x, edge_index, W, b):
    T_bs, NT, in_maps = _preprocess(x, edge_index, W, b)
    key = (T_bs, NT)
    if key not in _cache:
        _cache[key] = _build(T_bs, NT)
    nc = _cache[key]
    res = bass_utils.run_bass_kernel_spmd(nc, in_maps, core_ids=list(range(NC)))
    stacked = np.concatenate([res.results[c]["yout"] for c in range(NC)], axis=0)
    return stacked[_pos].astype(np.float32)


# revision 19
# speedup vs baseline: 1.0226x; 1.0104x over previous
"""GCNConv (N=20000, E=320000, D=1024) on 8 trn2 NeuronCores.

out = segment_sum(norm * h[col] -> row) with self-loops, h = x @ W^T + b
    = (segment_sum(norm * x[col] -> row)) @ W^T + s[row] * b,
      s[r] = sum of incoming norm (incl. self), norm = deg^-1/2 outer.

Aggregate-then-transform with an fp8 (e3m4) gather table: each core
stages the FULL x table pre-scaled to dis[c]*x[c]*2^k[c] (per-row pow2
normalization into e3m4's [7.75, 15.5] top range) in its local DRAM --
half the gather bytes of bf16, and e3m4's 4 mantissa bits keep the
end-to-end rel err ~1.3e-2.  Self loops ride along as ordinary edges
and duplicate sources within a (core, dest-block) are deduplicated, so
the selection matrices (built host-side, exact {1,2,...}*2^(kref-k)
values in e3m4) fold the whole normalization; the 2^-kref and dis[row]
factors fold into the PSUM->SBUF copy on the scalar engine.  Per dest
block (software-pipelined, gathers 2 blocks ahead, GEMM 1 block
behind): dma_gather source rows from the local fp8 table,
selection-matmul into fp32 PSUM, scaled-copy to fp16 on scalar,
PE-transpose z, then z @ W^T in fp16 with the bias applied on vector
as b*s + out.
"""

import numpy as np
import ml_dtypes

import concourse.bacc as bacc
import concourse.mybir as mybir
import concourse.tile as tile
from concourse import bass
from concourse import bass_utils
from concourse.masks import make_identity

N = 20000
E = 320000
D = 1024
NC = 8
NPC = N // NC            # 2500 real dest nodes per core
NBLK = 20                # dest blocks of 128 per core
NPCP = NBLK * 128        # 2560 padded dest nodes per core
P = 128
KT = D // P              # 8 contraction tiles of the out GEMM

_cache = {}
_pos = None

FP8 = ml_dtypes.float8_e3m4


def _preprocess(x, edge_index, W, b):
    x = np.asarray(x, dtype=np.float32)
    ei = np.asarray(edge_index)
    W = np.asarray(W, dtype=np.float32)
    b = np.asarray(b, dtype=np.float32)

    row = ei[0].astype(np.int64)
    col = ei[1].astype(np.int64)
    deg = (np.bincount(row, minlength=N) + 1).astype(np.float32)
    dis = deg ** -0.5

    # bias scale s[r] = total incoming weight incl. the dis^2 self loop
    wA = (dis[row] * dis[col]).astype(np.float64)
    s = (np.bincount(row, weights=wA, minlength=N) + dis * dis).astype(np.float32)

    # fp8 table: per-row pow2 normalization of v = dis[c] * x[c] into
    # e3m4's top binade; the inverse 2^(kref-k) goes into the selection
    # weights (exact in e3m4), the global 2^-kref into the z row scale
    v = dis[:, None] * x
    vmax = np.maximum(np.abs(v).max(axis=1), 1e-30)
    k = np.floor(np.log2(15.49 / vmax)).astype(np.int32)
    kref = int(k.max())
    xw8 = (v * (2.0 ** k)[:, None]).astype(FP8)
    selw = (2.0 ** (kref - k)).astype(np.float32)  # per-source sel weight

    # balance edge counts across the 160 (core, block) bins by dealing
    # nodes snake-wise in descending in-degree order: every bin ends up
    # with 125 nodes and an equal edge count, so the SPMD lockstep
    # schedule has no padding-tile overhead
    indeg = np.bincount(row, minlength=N)
    order_n = np.argsort(-indeg, kind="stable")
    NBIN = NC * NBLK
    bins = np.empty(N, dtype=np.int64)
    slot = np.empty(N, dtype=np.int64)
    fwd = np.arange(NBIN)
    for i in range(N // NBIN):
        selr = order_n[i * NBIN : (i + 1) * NBIN]
        bins[selr] = fwd if i % 2 == 0 else fwd[::-1]
        slot[selr] = i
    pos = (bins // NBLK) * NPCP + (bins % NBLK) * P + slot  # node -> padded row
    global _pos
    _pos = pos

    # append self loops as ordinary edges (weight folds the same way)
    rowA = np.concatenate([row, np.arange(N, dtype=np.int64)])
    colA = np.concatenate([col, np.arange(N, dtype=np.int64)])

    pdst = pos[rowA]
    core_of = pdst // NPCP
    within = pdst - core_of * NPCP
    blk = within // P
    dl_all = within % P  # dest offset in block

    # per (core, block) edge lists, sorted+deduped by source
    key = core_of * NBLK + blk
    order = np.argsort(key, kind="stable")
    ks = key[order]
    bounds = np.searchsorted(ks, np.arange(NC * NBLK + 1))
    colS, dlS = colA[order], dl_all[order]
    seg = {}
    cnt = np.zeros((NC, NBLK), dtype=np.int64)
    for c in range(NC):
        for bk in range(NBLK):
            i0, i1 = bounds[c * NBLK + bk], bounds[c * NBLK + bk + 1]
            srcs, sl = np.unique(colS[i0:i1], return_inverse=True)
            seg[(c, bk)] = (srcs, sl, dlS[i0:i1])
            cnt[c, bk] = len(srcs)

    # shared (SPMD lockstep) schedule: tiles per block = max over cores
    T_bs = tuple(int(-(-int(cnt[:, bk].max()) // P)) for bk in range(NBLK))
    NT = sum(T_bs)

    idx16 = np.zeros((NC, 16, NT * 8), dtype=np.int16)
    tstart = np.concatenate([[0], np.cumsum(T_bs)]).astype(np.int64)

    # GEMM rhs: 8 k-tiles of W^T; bias applied on vector as (b * s) + out
    WT8 = np.zeros((P, KT * D), dtype=np.float16)
    WTc = W.T.astype(np.float16)
    for kk in range(KT):
        WT8[:, kk * D : (kk + 1) * D] = WTc[kk * P : (kk + 1) * P, :]
    bbc = np.ascontiguousarray(
        np.broadcast_to(b.astype(np.float16), (P, D))
    )
    identm = np.eye(P, dtype=np.float16)

    # per-node tables scattered into permuted padded positions
    dis_all = np.zeros(NC * NPCP, dtype=np.float32)
    dis_all[pos] = dis * (2.0 ** (-kref))
    s_all = np.zeros(NC * NPCP, dtype=np.float32)
    s_all[pos] = s

    in_maps = []
    selmax = 0.0
    for c in range(NC):
        sel = np.zeros((P, NT, P), dtype=np.float32)
        for bk in range(NBLK):
            t0 = int(tstart[bk])
            srcs, sl, dl = seg[(c, bk)]
            n = len(srcs)
            # slot s -> partition s%128, tile t0 + s//128 (gather layout)
            np.add.at(sel, (sl % P, t0 + sl // P, dl), selw[srcs][sl])
            buf = np.zeros(T_bs[bk] * P, dtype=np.int16)
            buf[:n] = srcs.astype(np.int16)
            idx16[c, :, t0 * 8 : (t0 + T_bs[bk]) * 8] = (
                buf.reshape(T_bs[bk] * 8, 16).T
            )
        selmax = max(selmax, float(sel.max()))
        sel8 = sel.astype(FP8)
        disT = dis_all[c * NPCP : (c + 1) * NPCP].reshape(NBLK, P).T
        sT = s_all[c * NPCP : (c + 1) * NPCP].reshape(NBLK, P).T
        in_maps.append(
            {
                "xw8": xw8,
                "disT": np.ascontiguousarray(disT),
                "sT": np.ascontiguousarray(sT),
                "WT8": np.ascontiguousarray(WT8),
                "bbc": bbc,
                "identm": identm,
                "idx16": np.ascontiguousarray(np.tile(idx16[c], (8, 1))),
                "sel": np.ascontiguousarray(sel8.reshape(P, NT * P)),
            }
        )
    assert selmax <= 15.49, f"sel overflow {selmax}"
    return T_bs, NT, in_maps


def _build(T_bs, NT):
    f32 = mybir.dt.float32
    f16 = mybir.dt.float16
    f8 = mybir.dt.float8e3
    i16 = mybir.dt.int16
    tstart = {}
    t0 = 0
    for bk in range(NBLK):
        tstart[bk] = t0
        t0 += T_bs[bk]

    nc = bacc.Bacc("TRN2", target_bir_lowering=False, debug=False,
                   num_devices=NC, num_swdge_queues=4)
    xw8 = nc.dram_tensor("xw8", [N, D], f8, kind="ExternalInput").ap()
    disTi = nc.dram_tensor("disT", [P, NBLK], f32, kind="ExternalInput").ap()
    sTi = nc.dram_tensor("sT", [P, NBLK], f32, kind="ExternalInput").ap()
    WT8 = nc.dram_tensor("WT8", [P, KT * D], f16, kind="ExternalInput").ap()
    bbc = nc.dram_tensor("bbc", [P, D], f16, kind="ExternalInput").ap()
    identi = nc.dram_tensor("identm", [P, P], f16, kind="ExternalInput").ap()
    idx16 = nc.dram_tensor("idx16", [P, NT * 8], i16, kind="ExternalInput").ap()
    seli = nc.dram_tensor("sel", [P, NT * P], f8, kind="ExternalInput").ap()
    yout = nc.dram_tensor("yout", [NPCP, D], f16, kind="ExternalOutput").ap()

    chunks = [slice(s0, s0 + 512) for s0 in range(0, D, 512)]

    with tile.TileContext(nc) as tc:
        with tc.tile_pool(name="const", bufs=1) as const:
            # gather-critical consts load first so the pipeline can start;
            # everything the gemm/bias needs is deferred until after the
            # first gathers are in flight (see below)
            ix_sb = const.tile([P, NT * 8], i16, name="ix_sb")
            disT_sb = const.tile([P, NBLK], f32, name="disT_sb")
            ident = const.tile([P, P], f16, name="ident")
            wt_sb = const.tile([P, KT * D], f16, name="wt_sb")
            bbc_sb = const.tile([P, D], f16, name="bbc_sb")
            sT_sb = const.tile([P, NBLK], f32, name="sT_sb")

            def load_ix(bk, eng=None):
                # per-block index slices: gather(bk) waits only on its slice
                t0, Tb = tstart[bk], T_bs[bk]
                (eng or nc.sync).dma_start(ix_sb[:, t0 * 8 : (t0 + Tb) * 8],
                                           idx16[:, t0 * 8 : (t0 + Tb) * 8])

            def load_late_consts():
                # big consts ride the scalar HWDGE ring so they never sit
                # ahead of the gather-critical ix/sel loads on sync
                nc.scalar.dma_start(disT_sb[:], disTi[:])
                for k in range(KT):
                    nc.scalar.dma_start(wt_sb[:, k * D : (k + 1) * D],
                                        WT8[:, k * D : (k + 1) * D])
                nc.scalar.dma_start(bbc_sb[:], bbc[:])
                nc.scalar.dma_start(sT_sb[:], sTi[:])

            with tc.tile_pool(name="gath", bufs=3) as gp, \
                 tc.tile_pool(name="sel", bufs=4) as selp, \
                 tc.tile_pool(name="zps", bufs=2, space="PSUM") as zps, \
                 tc.tile_pool(name="tps", bufs=1, space="PSUM") as tps, \
                 tc.tile_pool(name="ops", bufs=1, space="PSUM") as ops, \
                 tc.tile_pool(name="zsb", bufs=3) as zsbp, \
                 tc.tile_pool(name="ztb", bufs=3) as ztbp, \
                 tc.tile_pool(name="aout", bufs=2) as aoutp:
                state = {"gq": 0}
                gs, sels, zds, zts = {}, {}, {}, {}

                # --- pipeline stages -------------------------------------
                def issue_sel(bk):
                    Tb, t0 = T_bs[bk], tstart[bk]
                    selb = selp.tile([P, Tb, P], f8, tag="selb")
                    nc.sync.dma_start(
                        selb[:], seli[:, t0 * P : (t0 + Tb) * P])
                    sels[bk] = selb

                def issue_gather(bk, splits=(8, 8)):
                    Tb, t0 = T_bs[bk], tstart[bk]
                    g = gp.tile([P, Tb, D], f8, tag="g")
                    bounds = [0]
                    for s in splits:
                        bounds.append(min(Tb, bounds[-1] + s))
                    while bounds[-1] < Tb:
                        bounds.append(min(Tb, bounds[-1] + 8))
                    for s0, s1 in zip(bounds, bounds[1:]):
                        sn = s1 - s0
                        if sn <= 0:
                            continue
                        nc.gpsimd.dma_gather(
                            g[:, s0 : s0 + sn, :],
                            xw8[:],
                            ix_sb[:, (t0 + s0) * 8 : (t0 + s0 + sn) * 8],
                            sn * P,
                            sn * P,
                            D,
                            queue_num=state["gq"],
                            single_packet=True,
                        )
                        state["gq"] = (state["gq"] + 1) % 4
                    gs[bk] = g

                def issue_agg(bk):
                    # z[d, f] = sum over tiles of sel^T @ g, chunk-outer so
                    # each 512-col half finishes early and its scaled fp16
                    # copy (scalar engine, folds dis[dest] * 2^-kref) hides
                    # under the other half's matmuls
                    Tb = T_bs[bk]
                    g, selb = gs.pop(bk), sels.pop(bk)
                    halves = []
                    for ci, cs in enumerate(chunks):
                        zp = zps.tile([P, 512], f32, tag=f"zp{ci}")
                        for i in range(Tb):
                            nc.tensor.matmul(
                                zp[:], lhsT=selb[:, i, :], rhs=g[:, i, cs],
                                start=(i == 0), stop=(i == Tb - 1),
                            )
                        zsb = zsbp.tile([P, 512], f16, tag=f"zsb{ci}")
                        if ci == 0:
                            # chunk 0 copy hides under chunk 1's matmuls
                            nc.scalar.activation(
                                out=zsb[:], in_=zp[:],
                                func=mybir.ActivationFunctionType.Copy,
                                scale=disT_sb[:, bk : bk + 1],
                            )
                        else:
                            # chunk 1 is on the critical path to the
                            # transposes -- vector is faster
                            nc.vector.tensor_scalar_mul(
                                out=zsb[:], in0=zp[:],
                                scalar1=disT_sb[:, bk : bk + 1],
                            )
                        halves.append(zsb)
                    zds[bk] = halves

                def issue_transpose(bk):
                    # transpose z into 8 k-tiles [f, d] on the PE, in two
                    # halves so the zt copies overlap the other half
                    zh = zds.pop(bk)
                    parts = []
                    for ci in range(2):
                        tp = tps.tile([P, KT // 2, P], f16, tag=f"tp{ci}")
                        for ft in range(KT // 2):
                            nc.tensor.transpose(
                                tp[:, ft, :],
                                zh[ci][:, ft * P : (ft + 1) * P],
                                ident[:])
                        zt = ztbp.tile([P, KT // 2, P], f16, tag=f"zt{ci}")
                        nc.vector.tensor_copy(zt[:], tp[:])
                        parts.append(zt)
                    zts[bk] = parts

                def issue_gemm(bk):
                    # out = z @ W^T, chunk-outer so each 512-col half gets
                    # its bias (vector: ob = b * s + out) and store while
                    # the other half's matmuls run
                    zt2 = zts.pop(bk)
                    for ci, cs in enumerate(chunks):
                        op_ = ops.tile([P, 512], f32, tag=f"op{ci}")
                        for k in range(KT):
                            zt = zt2[k // 4]
                            nc.tensor.matmul(
                                op_[:], lhsT=zt[:, k % 4, :],
                                rhs=wt_sb[:, k * D + cs.start : k * D + cs.stop],
                                start=(k == 0), stop=(k == KT - 1),
                            )
                        ob = aoutp.tile([P, 512], f16, tag=f"ob{ci}")
                        nc.vector.scalar_tensor_tensor(
                            out=ob[:], in0=bbc_sb[:, cs],
                            scalar=sT_sb[:, bk : bk + 1],
                            in1=op_[:], op0=mybir.AluOpType.mult,
                            op1=mybir.AluOpType.add,
                        )
                        nc.sync.dma_start(yout[bk * P : (bk + 1) * P, cs], ob[:])

                # --- software pipeline: gathers/sels lead, gemm trails by
                # two blocks and transposes by one so every PSUM-evacuation
                # copy has a full block of slack.  The first gather call is
                # tiny so agg(0) can start the moment its first tile lands
                # (the first DMAGatherAnt also pays a ~15us gpsimd library
                # reload, so everything else stays off the Pool sequencer).
                nc.sync.dma_start(ident[:], identi[:])
                load_ix(0)
                load_ix(1)
                issue_sel(0)
                issue_sel(1)
                issue_gather(0, splits=(1, 3, 4, 8))
                issue_gather(1)
                load_late_consts()
                # keep the PE HAM clock-gate warm while gathers land
                warm = zps.tile([P, 512], f32, tag="zp0", name="warm")
                for _ in range(80):
                    nc.tensor.matmul(warm[:, :P], lhsT=ident[:], rhs=ident[:],
                                     start=True, stop=True)
                for bk in range(NBLK):
                    if bk + 2 < NBLK:
                        load_ix(bk + 2)
                        issue_sel(bk + 2)
                        issue_gather(bk + 2)
                    issue_agg(bk)
                    if bk >= 2:
                        issue_gemm(bk - 2)
                    if bk >= 1:
                        issue_transpose(bk - 1)
                issue_gemm(NBLK - 2)
                issue_transpose(NBLK - 1)
                issue_gemm(NBLK - 1)

    nc.compile()
    return nc


def kernel(x, edge_index, W, b):
    T_bs, NT, in_maps = _preprocess(x, edge_index, W, b)
    key = (T_bs, NT)
    if key not in _cache:
        _cache[key] = _build(T_bs, NT)
    nc = _cache[key]
    res = bass_utils.run_bass_kernel_spmd(nc, in_maps, core_ids=list(range(NC)))
    stacked = np.concatenate([res.results[c]["yout"] for c in range(NC)], axis=0)
    return stacked[_pos].astype(np.float32)
